# revision 2
# baseline (speedup 1.0000x reference)
"""Trainium2 kernel for nn_DepthModule (multi-view stereo depth head).

Contract: kernel(**inputs) takes FULL unsharded numpy inputs and returns the
FULL output [1, 60, 80] float32 depth map.

Distribution strategy (8 NeuronCores, per the sharding hint):
  - The 32-bin depth-hypothesis axis D of the cost volume is sharded across
    the 8 cores: each core owns a 4-deep logits slab.
  - Each core warps/samples its own depth slab (plus a 2-deep halo on each
    side, recomputed locally instead of exchanged) from fmaps it encodes
    itself, so the 3D decoder needs no inter-core halo exchange.
  - One all-gather of the [4, 60, 80] logits slabs reassembles the full
    [32, 60, 80] logits tensor on every core; SoftArgmax runs replicated.

Everything runs on the NeuronCores through the PJRT backend; the host only
stages inputs and slices out the final replicated output.
"""

import functools

import numpy as np

# Hardcoded problem geometry (must not read spec/reference at grade time).
HT, WD = 480, 640
NDEPTH = 32
FRAMES = 5
MIN_DEPTH, MAX_DEPTH = 0.25, 8.0
N_CORES = 8
D_SLAB = NDEPTH // N_CORES  # 4 logits depths per core
HALO = 2                    # two 3x3x3 convs -> need vol slab of D_SLAB + 2*2


def _conv2d(x, w, b, s):
    import jax
    y = jax.lax.conv_general_dilated(
        x, w, (s, s), 'SAME', dimension_numbers=('NCHW', 'OIHW', 'NCHW'))
    return y + b[None, :, None, None]


def _conv3d_valid_d(x, w, b):
    """3D conv, VALID along depth, SAME along h/w. x: [N,C,D,H,W]."""
    import jax
    y = jax.lax.conv_general_dilated(
        x, w, (1, 1, 1), [(0, 0), (1, 1), (1, 1)],
        dimension_numbers=('NCDHW', 'OIDHW', 'NCDHW'))
    return y + b[None, :, None, None, None]


def _bilinear_sample(fmap, u, v):
    import jax.numpy as jnp
    C, h, w = fmap.shape
    x0 = jnp.floor(u); y0 = jnp.floor(v)
    wx = u - x0; wy = v - y0
    x0i = x0.astype(jnp.int32); y0i = y0.astype(jnp.int32)

    def gather(yi, xi):
        yc = jnp.clip(yi, 0, h - 1); xc = jnp.clip(xi, 0, w - 1)
        return fmap[:, yc, xc]

    val = (gather(y0i, x0i) * (1 - wx) * (1 - wy)
           + gather(y0i, x0i + 1) * wx * (1 - wy)
           + gather(y0i + 1, x0i) * (1 - wx) * wy
           + gather(y0i + 1, x0i + 1) * wx * wy)
    valid = (u >= 0) & (u <= w - 1) & (v >= 0) & (v <= h - 1)
    return val * valid[None, :].astype(fmap.dtype)


def _intrinsics_matrix(v):
    import jax.numpy as jnp
    fx, fy, cx, cy = v[:, 0], v[:, 1], v[:, 2], v[:, 3]
    z = jnp.zeros_like(fx); o = jnp.ones_like(fx)
    return jnp.stack([fx, z, cx, z, fy, cy, z, z, o], axis=-1).reshape(-1, 3, 3)


def _per_core(core_idx, poses, images, intrinsics,
              w1, b1, w2, b2, w3, b3, wd1, bd1, wd2, bd2):
    """Body executed on every core under pmap. Returns replicated [1,60,80]."""
    import jax
    import jax.numpy as jnp

    B, F, _, ht, wd = images.shape
    x = 2.0 * (images / 255.0) - 1.0
    x = x.reshape(B * F, 3, ht, wd)
    x = jax.nn.relu(_conv2d(x, w1, b1, 2))
    x = jax.nn.relu(_conv2d(x, w2, b2, 2))
    x = jax.nn.relu(_conv2d(x, w3, b3, 2))
    h, w = ht // 8, wd // 8
    fmaps = x.reshape(B, F, 32, h, w)

    depths_full = jnp.linspace(MIN_DEPTH, MAX_DEPTH, NDEPTH).astype(images.dtype)
    # This core's vol slab covers global depth rows [lo, lo + D_SLAB + 2*HALO).
    lo = core_idx * D_SLAB - HALO
    d_idx = lo + jnp.arange(D_SLAB + 2 * HALO)          # [8] global d rows
    d_valid = (d_idx >= 0) & (d_idx < NDEPTH)
    depths = depths_full[jnp.clip(d_idx, 0, NDEPTH - 1)]

    Kv = intrinsics / 4.0
    K = _intrinsics_matrix(Kv)
    ys, xs = jnp.meshgrid(jnp.arange(h, dtype=x.dtype),
                          jnp.arange(w, dtype=x.dtype), indexing='ij')
    pix = jnp.stack([xs.ravel(), ys.ravel(), jnp.ones(h * w, x.dtype)], 0)
    # closed-form inverse of the intrinsics matrix (triangular-solve is not
    # supported by the neuron compiler)
    fx, fy, cx, cy = Kv[:, 0], Kv[:, 1], Kv[:, 2], Kv[:, 3]
    z0 = jnp.zeros_like(fx); o0 = jnp.ones_like(fx)
    Kinv = jnp.stack([1 / fx, z0, -cx / fx,
                      z0, 1 / fy, -cy / fy,
                      z0, z0, o0], axis=-1).reshape(-1, 3, 3)
    rays = jnp.einsum('bij,jn->bin', Kinv, pix)
    pts = depths[None, :, None, None] * rays[:, None]            # [B,Ds,3,hw]
    # closed-form rigid inverse of the keyframe pose
    R0 = poses[:, 0, :3, :3]; t0 = poses[:, 0, :3, 3]
    R0T = jnp.swapaxes(R0, -1, -2)
    it = -jnp.einsum('bij,bj->bi', R0T, t0)
    top = jnp.concatenate([R0T, it[:, :, None]], axis=-1)        # [B,3,4]
    bot = jnp.tile(jnp.array([[[0., 0., 0., 1.]]], x.dtype), (top.shape[0], 1, 1))
    pose0_inv = jnp.concatenate([top, bot], axis=1)              # [B,4,4]
    G = jnp.einsum('bfij,bjk->bfik', poses, pose0_inv)
    X = jnp.einsum('bfij,bdjn->bfdin', G[..., :3, :3], pts) \
        + G[..., :3, 3][:, :, None, :, None]
    proj = jnp.einsum('bij,bfdjn->bfdin', K, X)
    z = proj[:, :, :, 2]
    u = proj[:, :, :, 0] / (z + 1e-8)
    v = proj[:, :, :, 1] / (z + 1e-8)
    Ds = D_SLAB + 2 * HALO
    sample = jax.vmap(jax.vmap(_bilinear_sample))                # over B, F
    warped = sample(fmaps, u.reshape(B, F, -1), v.reshape(B, F, -1))
    warped = warped.reshape(B, F, 32, Ds, h, w)
    avg = warped.mean(axis=1)
    ref = jnp.broadcast_to(fmaps[:, 0][:, :, None], (B, 32, Ds, h, w))
    vol = jnp.concatenate([ref, avg], axis=1)                    # [B,64,Ds,h,w]
    # Depth rows outside [0, NDEPTH) are the 3D conv's zero padding.
    vol = vol * d_valid[None, None, :, None, None].astype(vol.dtype)

    h3 = jax.nn.relu(_conv3d_valid_d(vol, wd1, bd1))             # rows lo+1..lo+7
    # Zero h3 rows whose global depth index is out of range (conv SAME pad).
    h3_idx = d_idx[1:-1]
    h3 = h3 * ((h3_idx >= 0) & (h3_idx < NDEPTH))[None, None, :, None, None].astype(h3.dtype)
    logits_slab = _conv3d_valid_d(h3, wd2, bd2)[:, 0]            # [B,4,h,w]

    slabs = jax.lax.all_gather(logits_slab, 'x', axis=0)         # [8,B,4,h,w]
    logits = jnp.moveaxis(slabs, 0, 1).reshape(B, NDEPTH, h, w)
    prob = jax.nn.softmax(logits, axis=1).transpose(0, 2, 3, 1)
    return jnp.sum(depths_full * prob, axis=-1)                  # [B,h,w]


@functools.cache
def _compiled():
    import jax
    devs = jax.devices()[:N_CORES]
    fn = jax.pmap(_per_core, axis_name='x', devices=devs,
                  in_axes=(0,) + (None,) * 13)
    return fn


def kernel(poses, images, intrinsics, w1, b1, w2, b2, w3, b3,
           wd1, bd1, wd2, bd2):
    fn = _compiled()
    core_ids = np.arange(N_CORES, dtype=np.int32)
    out = fn(core_ids, poses, images, intrinsics,
             w1, b1, w2, b2, w3, b3, wd1, bd1, wd2, bd2)
    return np.asarray(out[0]).astype(np.float32)     # replicated -> core 0


# revision 5
# speedup vs baseline: 1.4839x; 1.4839x over previous
"""v2: frame-sharded encoder + matrix-form separable bilinear warp.

Sharding (8 NeuronCores):
  stage 1: core f encodes frame f (frames padded 5->8); all-gather fmaps.
  stage 2: core c warps its 4-deep depth slab (+2 halo each side, recomputed
           locally) and runs the 3D decoder on it; all-gather of the 4-deep
           logits slabs; SoftArgmax replicated.

The bilinear warp is expressed as two small dense interpolation matmuls
(Wy @ fmap @ Wx^T) instead of a per-pixel gather — exact for poses whose
rotation block is the identity (true for this problem's pose distribution,
checked on host; general poses fall back to the gather implementation).
"""

import functools

import numpy as np

HT, WD = 480, 640
NDEPTH = 32
FRAMES = 5
MIN_DEPTH, MAX_DEPTH = 0.25, 8.0
N_CORES = 8
D_SLAB = NDEPTH // N_CORES
HALO = 2


def _conv2d(x, w, b, s):
    import jax
    y = jax.lax.conv_general_dilated(
        x, w, (s, s), 'SAME', dimension_numbers=('NCHW', 'OIHW', 'NCHW'))
    return y + b[None, :, None, None]


def _conv3d_valid_d(x, w, b):
    import jax
    y = jax.lax.conv_general_dilated(
        x, w, (1, 1, 1), [(0, 0), (1, 1), (1, 1)],
        dimension_numbers=('NCDHW', 'OIDHW', 'NCDHW'))
    return y + b[None, :, None, None, None]


def _bilinear_sample(fmap, u, v):
    import jax.numpy as jnp
    C, h, w = fmap.shape
    x0 = jnp.floor(u); y0 = jnp.floor(v)
    wx = u - x0; wy = v - y0
    x0i = x0.astype(jnp.int32); y0i = y0.astype(jnp.int32)

    def gather(yi, xi):
        yc = jnp.clip(yi, 0, h - 1); xc = jnp.clip(xi, 0, w - 1)
        return fmap[:, yc, xc]

    val = (gather(y0i, x0i) * (1 - wx) * (1 - wy)
           + gather(y0i, x0i + 1) * wx * (1 - wy)
           + gather(y0i + 1, x0i) * (1 - wx) * wy
           + gather(y0i + 1, x0i + 1) * wx * wy)
    valid = (u >= 0) & (u <= w - 1) & (v >= 0) & (v <= h - 1)
    return val * valid[None, :].astype(fmap.dtype)


def _per_core(core_idx, frame, poses, intrinsics,
              w1, b1, w2, b2, w3, b3, wd1, bd1, wd2, bd2, use_matrix_warp):
    import jax
    import jax.numpy as jnp

    # ---- stage 1: encode this core's frame, all-gather fmaps ----
    # frame arrives as f16 (transfer over the host link is the dominant
    # cost; f16 halves it at ~5e-4 input rounding error)
    x = 2.0 * (frame[None].astype(jnp.float32) / 255.0) - 1.0  # [1,3,H,W]
    x = jax.nn.relu(_conv2d(x, w1, b1, 2))
    x = jax.nn.relu(_conv2d(x, w2, b2, 2))
    x = jax.nn.relu(_conv2d(x, w3, b3, 2))
    h, w = HT // 8, WD // 8
    fmaps_all = jax.lax.all_gather(x[0], 'x', axis=0)          # [8,32,h,w]
    fmaps = fmaps_all[:FRAMES][None]                           # [1,5,32,h,w]

    B, F = 1, FRAMES
    dtype = x.dtype
    depths_full = jnp.linspace(MIN_DEPTH, MAX_DEPTH, NDEPTH).astype(dtype)
    lo = core_idx * D_SLAB - HALO
    d_idx = lo + jnp.arange(D_SLAB + 2 * HALO)
    d_valid = (d_idx >= 0) & (d_idx < NDEPTH)
    depths = depths_full[jnp.clip(d_idx, 0, NDEPTH - 1)]
    Ds = D_SLAB + 2 * HALO

    # ---- projection geometry (closed-form inverses) ----
    Kv = intrinsics / 4.0
    fx, fy, cx, cy = Kv[:, 0], Kv[:, 1], Kv[:, 2], Kv[:, 3]
    z0 = jnp.zeros_like(fx); o0 = jnp.ones_like(fx)
    K = jnp.stack([fx, z0, cx, z0, fy, cy, z0, z0, o0], -1).reshape(-1, 3, 3)
    Kinv = jnp.stack([1 / fx, z0, -cx / fx, z0, 1 / fy, -cy / fy,
                      z0, z0, o0], -1).reshape(-1, 3, 3)
    R0 = poses[:, 0, :3, :3]; t0 = poses[:, 0, :3, 3]
    R0T = jnp.swapaxes(R0, -1, -2)
    it = -jnp.einsum('bij,bj->bi', R0T, t0)
    top = jnp.concatenate([R0T, it[:, :, None]], axis=-1)
    bot = jnp.tile(jnp.array([[[0., 0., 0., 1.]]], dtype), (top.shape[0], 1, 1))
    pose0_inv = jnp.concatenate([top, bot], axis=1)
    G = jnp.einsum('bfij,bjk->bfik', poses, pose0_inv)

    ys, xs = jnp.meshgrid(jnp.arange(h, dtype=dtype),
                          jnp.arange(w, dtype=dtype), indexing='ij')
    pix = jnp.stack([xs.ravel(), ys.ravel(), jnp.ones(h * w, dtype)], 0)
    rays = jnp.einsum('bij,jn->bin', Kinv, pix)
    pts = depths[None, :, None, None] * rays[:, None]
    X = jnp.einsum('bfij,bdjn->bfdin', G[..., :3, :3], pts) \
        + G[..., :3, 3][:, :, None, :, None]
    proj = jnp.einsum('bij,bfdjn->bfdin', K, X)
    z = proj[:, :, :, 2]
    u = proj[:, :, :, 0] / (z + 1e-8)
    v = proj[:, :, :, 1] / (z + 1e-8)

    fm5 = fmaps[0]                                             # [5,32,h,w]
    if use_matrix_warp:
        # u constant along rows, v constant along cols (identity rotation):
        # bilinear == Wy @ fmap @ Wx^T with triangular row/col weights.
        u_r = u.reshape(B, F, Ds, h, w)[0, :, :, 0, :]         # [F,Ds,w]
        v_c = v.reshape(B, F, Ds, h, w)[0, :, :, :, 0]         # [F,Ds,h]
        xg = jnp.arange(w, dtype=dtype)
        yg = jnp.arange(h, dtype=dtype)
        Wx = jax.nn.relu(1.0 - jnp.abs(u_r[..., None] - xg))   # [F,Ds,w,w] (j,x)
        Wx = Wx * ((u_r >= 0) & (u_r <= w - 1))[..., None].astype(dtype)
        Wy = jax.nn.relu(1.0 - jnp.abs(v_c[..., None] - yg))   # [F,Ds,h,y]
        Wy = Wy * ((v_c >= 0) & (v_c <= h - 1))[..., None].astype(dtype)
        t1 = jnp.einsum('fcyx,fdjx->fcdyj', fm5, Wx)
        warped = jnp.einsum('fdiy,fcdyj->fcdij', Wy, t1)       # [F,32,Ds,h,w]
        avg = warped.mean(axis=0)[None]                        # [1,32,Ds,h,w]
    else:
        sample = jax.vmap(jax.vmap(_bilinear_sample))
        warped = sample(fmaps, u.reshape(B, F, -1), v.reshape(B, F, -1))
        warped = warped.reshape(B, F, 32, Ds, h, w)
        avg = warped.mean(axis=1)

    ref = jnp.broadcast_to(fm5[0][None, :, None], (B, 32, Ds, h, w))
    vol = jnp.concatenate([ref, avg], axis=1)
    vol = vol * d_valid[None, None, :, None, None].astype(vol.dtype)

    h3 = jax.nn.relu(_conv3d_valid_d(vol, wd1, bd1))
    h3_idx = d_idx[1:-1]
    h3 = h3 * ((h3_idx >= 0) & (h3_idx < NDEPTH))[None, None, :, None, None].astype(h3.dtype)
    logits_slab = _conv3d_valid_d(h3, wd2, bd2)[:, 0]          # [1,4,h,w]

    slabs = jax.lax.all_gather(logits_slab, 'x', axis=0)
    logits = jnp.moveaxis(slabs, 0, 1).reshape(B, NDEPTH, h, w)
    prob = jax.nn.softmax(logits, axis=1).transpose(0, 2, 3, 1)
    return jnp.sum(depths_full * prob, axis=-1)


@functools.cache
def _compiled(use_matrix_warp):
    import jax
    devs = jax.devices()[:N_CORES]
    fn = functools.partial(_per_core, use_matrix_warp=use_matrix_warp)
    return jax.pmap(fn, axis_name='x', devices=devs,
                    in_axes=(0, 0) + (None,) * 12)


def kernel(poses, images, intrinsics, w1, b1, w2, b2, w3, b3,
           wd1, bd1, wd2, bd2):
    # matrix-form warp is exact iff every relative rotation is the identity
    R = np.asarray(poses)[0, :, :3, :3]
    use_matrix = bool(np.all(np.abs(R - np.eye(3, dtype=R.dtype)) == 0))
    frames = np.zeros((N_CORES, 3, HT, WD), np.float16)
    frames[:FRAMES] = np.asarray(images)[0].astype(np.float16)
    fn = _compiled(use_matrix)
    core_ids = np.arange(N_CORES, dtype=np.int32)
    out = fn(core_ids, frames, poses, intrinsics,
             w1, b1, w2, b2, w3, b3, wd1, bd1, wd2, bd2)
    return np.asarray(out[0]).astype(np.float32)
